# revision 42
# baseline (speedup 1.0000x reference)
"""Causal single-head attention (B=8, T=4096, C=1024, H=128) on 8 TRN2
NeuronCores, data-parallel over batch: core b computes batch element b.

Host pre-transposes and pre-casts: each core gets xT [C, T] fp16 (so the
contraction dim is already on partitions -- no on-chip transposes) plus
Wq/Wk/Wv [C, H] fp16 replicated. Output is [T, H] f32.

The TensorE stream keeps a chunky emission style (long uninterrupted
matmul runs minimize per-instruction sync overhead and keep LDWEIGHTS
pulled ahead -- fine-grained interleaving measurably trades idle time
for equal per-MM overhead), with the trace-measured loss sources fixed:
 - startup: junk-matmul warmup spans the first DMA wait (so the HAM
   clock gate reaches 8/8 before real work), first weight/x slices use
   >=512B DMA lines split across both HWDGE queues, and chunk 0 runs
   q/k projection -> score pairs immediately (its v-projection and
   chunk 1's work are deferred), so the first exp issues ~6us earlier
   than a phase-ordered schedule -- and warm.
 - the score pairs own a 3-deep PSUM ring (6 banks) so the exp spine
   always has up to two banked pairs of lookahead; projections, AV
   accumulators and the warmup complete inside their own emission
   blocks and time-share one double-buffered single-bank pool (2
   banks).  With a shared ring, a projection allocation shrinks the
   score lookahead and serializes exp(p) -> scores(p+1) -> exp(p+1),
   which was the dominant hidden cost of the original schedule.
 - every filler block is kept under ~2us (one projection, half a
   v-projection, or one AV query-block group) and hung on fixed hook
   points between score pairs; late chunks defer av(qb=1..3) into the
   next chunk so the exp-bound stretch always has PE work.
 - x chunks prefetch 2-3 ahead in halves on the Sync queue (weights
   ride the Scalar queue, fully ahead of any x there).
 - diagonal-pair exps cover only the causally needed ranges (junk left
   of the diagonal is never read); diagonal masking runs on the
   otherwise-idle GpSimd engine so the DVE cast/normalize queue stays
   out of the exp->AV chain.
Measured on-device: 157.4us (phase-ordered baseline) -> ~131-133us.
"""
import numpy as np

import concourse.bass as bass
import concourse.mybir as mybir
import concourse.tile as tile
from concourse.bass import ts
from contextlib import ExitStack

F16 = mybir.dt.float16
F32 = mybir.dt.float32

B, T, C, H = 8, 4096, 1024, 128

# ---------------------------------------------------------------------------
# Workaround for the walrus build in this container: each TPB instruction may
# carry at most ONE sync-wait ("Too many sync wait commands" otherwise), but
# Tile attaches several. Keep only the last wait per instruction and hoist the
# others onto preceding same-engine NoOps (engines execute their stream in
# order, so the gating semantics are identical). The tail drain gets the same
# treatment.
# ---------------------------------------------------------------------------
_MAX_WAITS = 1
_orig_add_instruction = tile.TileContext._add_instruction


def _split_waits_add_instruction(self, inst):
    si = inst.sync_info
    if (
        si is not None
        and len(si.on_wait) > _MAX_WAITS
        and inst.engine != mybir.EngineType.Unassigned
    ):
        waits = list(si.on_wait)
        extra, keep = waits[:-_MAX_WAITS], waits[-_MAX_WAITS:]
        for w in extra:
            nop = mybir.InstNoOp(
                name=self.nc.get_next_instruction_name(),
                engine=inst.engine,
                ins=[],
                outs=[],
                bass_nofuse=True,
                sync_info=mybir.SyncInfo(on_wait=[w], on_update=[]),
                debug=inst.debug,
            )
            _orig_add_instruction(self, nop)
        inst.sync_info = mybir.SyncInfo(on_wait=keep, on_update=list(si.on_update))
    return _orig_add_instruction(self, inst)


def _split_drain_and_barrier(self, tick_clock, wait_clock):
    nc = self.nc
    probe = nc.sync.nop(nofuse=True, hint="tile_drain_wait_split")
    wait_clock.add_sem_waits(
        probe.ins, tile.ScopedClock({None: tick_clock.global_clock})
    )
    si = probe.ins.sync_info
    waits = list(si.on_wait) if si is not None else []
    if len(waits) > _MAX_WAITS:
        probe.ins.sync_info = mybir.SyncInfo(
            on_wait=waits[:_MAX_WAITS], on_update=list(si.on_update)
        )
        rest = waits[_MAX_WAITS:]
        for i in range(0, len(rest), _MAX_WAITS):
            extra = nc.sync.nop(nofuse=True, hint=f"tile_drain_wait_split_{i}")
            extra.ins.sync_info = mybir.SyncInfo(
                on_wait=rest[i : i + _MAX_WAITS], on_update=[]
            )
    nc.sync.drain()
    nc.all_engine_barrier()
    assert self.sems is not None
    popped = nc._tile_sem_poison_stack.pop()
    assert popped is self._sem_poison
    nc.clear_and_free_semaphores(list(self.sems.allocated().values()))
    nc.all_engine_barrier()


def _apply_tile_patch():
    tile.TileContext._drain_and_barrier = _split_drain_and_barrier
    tile.TileContext._add_instruction = _split_waits_add_instruction


# ---------------------------------------------------------------------------
# Kernel builder
# ---------------------------------------------------------------------------
def build_attention(dtype=F16):
    TB = T // 128
    CB = C // 128
    NCH = T // 512
    scale = float(H) ** -0.5

    nc = bass.Bass()
    # host pre-arranged layouts: every DMA line is contiguous per partition
    # xt[c_chunk, ci, cb*512 + t] = x[t_global, cb*128 + ci]
    xt = nc.dram_tensor("xt", [NCH, 128, CB * 512], F16, kind="ExternalInput")
    # w*[ci, cb*H + h] = W[cb*128 + ci, h]
    wq = nc.dram_tensor("wq", [128, CB * H], F16, kind="ExternalInput")
    wk = nc.dram_tensor("wk", [128, CB * H], F16, kind="ExternalInput")
    wv = nc.dram_tensor("wv", [128, CB * H], F16, kind="ExternalInput")
    out = nc.dram_tensor("out", [T, H], F32, kind="ExternalOutput")

    with tile.TileContext(nc) as tc, ExitStack() as ctx:
        const = ctx.enter_context(tc.tile_pool(name="const", bufs=1))
        xsb = ctx.enter_context(tc.tile_pool(name="xsb", bufs=4))
        persist = ctx.enter_context(tc.tile_pool(name="persist", bufs=1))
        pP = ctx.enter_context(tc.tile_pool(name="pP", bufs=24))
        osb = ctx.enter_context(tc.tile_pool(name="osb", bufs=4))
        # Score pairs own a 3-deep ring (6 banks): the exp spine can bank two
        # pairs, absorbing any <=2us PE block between pair emissions.
        # Projections, AV accumulators and the warmup complete within their
        # own emission blocks, so they time-share one double-buffered
        # single-bank pool (2 banks) instead of holding dedicated banks.
        pps = ctx.enter_context(tc.tile_pool(name="pps", bufs=3, space="PSUM"))
        shr = ctx.enter_context(tc.tile_pool(name="shr", bufs=2, space="PSUM"))

        # --- PE warmup: the first input DMA cannot complete before ~12us
        # (engine preambles ~7us, then ~5us trigger-to-completion), so the
        # TensorE would idle >3.4us and the HAM clock gate would hold the
        # first ~16 real matmuls at 1.2GHz.  12 junk N=512 matmuls (~8 cold
        # + 4 warm ~= 4.3us) span the wait and hand the real projection a
        # warm (2.4GHz) array.
        warm16 = const.tile([128, 128], dtype, tag="warm16")
        warmS = const.tile([128, 512], dtype, tag="warmS")
        nc.gpsimd.memset(warm16[:], 0.0)
        nc.gpsimd.memset(warmS[:], 0.0)
        wps = shr.tile([128, 512], F32, tag="shr", name="warmup_ps")
        for i in range(10):
            nc.tensor.matmul(
                wps[:], warm16[:], warmS[:], start=True, stop=True,
            )

        # --- weights on the Scalar HWDGE queue, fully ahead of any x
        # slices there; first slices of wq/wk cover 2 cb blocks (512B
        # per-partition lines, the SDMA read-modify-write threshold).
        wsrc = {"q": wq, "k": wk, "v": wv}
        w16 = {}
        for name in ("q", "k", "v"):
            w16[name] = const.tile(
                [128, CB, H], dtype, tag=f"w{name}", name=f"w16{name}"
            )

        def load_w(name, lo, hi):
            nc.scalar.dma_start(
                w16[name][:, lo:hi, :],
                wsrc[name][:, lo * H : hi * H].rearrange(
                    "ci (cb h) -> ci cb h", cb=hi - lo
                ),
            )

        load_w("q", 0, 2)
        load_w("k", 0, 2)
        load_w("q", 2, CB)
        load_w("k", 2, CB)

        # mask16[jl, ql] = 1 if ql >= jl else 0 (transposed-score layout)
        mask16 = const.tile([128, 128], dtype, tag="mask")
        nc.gpsimd.memset(mask16[:], 1.0)
        nc.gpsimd.affine_select(
            out=mask16[:], in_=mask16[:],
            compare_op=mybir.AluOpType.is_ge,
            fill=0.0, base=0, pattern=[[1, 128]], channel_multiplier=-1,
        )

        qT16 = persist.tile([128, T], dtype, tag="qT")
        kT16 = persist.tile([128, T], dtype, tag="kT")
        v16 = persist.tile([128, TB, H + 1], dtype, tag="v")
        nc.gpsimd.memset(v16[:, :, H : H + 1], 1.0)  # ones column -> denominators

        x16s = {}

        def load_chunk(c):
            x16 = xsb.tile([128, CB, 512], dtype, tag="x16", name=f"x16_c{c}")
            x16s[c] = x16

            def dma(eng, lo, hi):
                eng.dma_start(
                    x16[:, lo:hi, :],
                    xt[c, :, lo * 512 : hi * 512].rearrange(
                        "ci (cb t) -> ci cb t", cb=hi - lo
                    ),
                )
            if c == 0:
                dma(nc.sync, 0, 1)
                dma(nc.sync, 1, 2)
                dma(nc.sync, 2, 4)
                dma(nc.scalar, 4, 6)
                dma(nc.sync, 6, 8)
            elif c == 1:
                dma(nc.sync, 0, 2)
                dma(nc.sync, 2, 4)
                dma(nc.scalar, 4, 8)
            else:
                dma(nc.sync, 0, 4)
                dma(nc.sync, 4, CB)

        def proj1(c, name):
            # one projection (q or k) for one chunk: ~1.8us block.  The
            # PSUM->SBUF cast runs on GpSimd: it is the shared-ring reader
            # that gates the ring's reuse, and the DVE queue (normalize +
            # v-casts) would often start it late.
            pja = shr.tile([128, 512], F32, tag="shr", name=f"pj{name}_{c}")
            x16 = x16s[c]
            for cb in range(CB):
                nc.tensor.matmul(
                    pja[:], w16[name][:, cb, :], x16[:, cb, :],
                    start=(cb == 0), stop=(cb == CB - 1),
                )
            dst = qT16 if name == "q" else kT16
            nc.vector.tensor_copy(dst[:, c * 512 : c * 512 + 512], pja[:])

        def qk_proj_interleaved(c):
            # chunk-0 startup path: q and k accumulate into two shared-pool
            # tiles with the cb loop outermost, matching DMA arrival order.
            pjq = shr.tile([128, 512], F32, tag="shr", name=f"pjq_{c}")
            pjk = shr.tile([128, 512], F32, tag="shr", name=f"pjk_{c}")
            x16 = x16s[c]
            for cb in range(CB):
                for name, pja in (("q", pjq), ("k", pjk)):
                    nc.tensor.matmul(
                        pja[:], w16[name][:, cb, :], x16[:, cb, :],
                        start=(cb == 0), stop=(cb == CB - 1),
                    )
            nc.vector.tensor_copy(qT16[:, c * 512 : c * 512 + 512], pjq[:])
            nc.vector.tensor_copy(kT16[:, c * 512 : c * 512 + 512], pjk[:])

        def v_proj(c, tbs):
            # two token blocks: ~1.1us block
            pja = shr.tile([128, 512], F32, tag="shr", name=f"pjv_{c}_{tbs[0]}")
            x16 = x16s[c]
            for i, tb in enumerate(tbs):
                for cb in range(CB):
                    nc.tensor.matmul(
                        pja[:, i * 256 : i * 256 + 128],
                        x16[:, cb, ts(tb, 128)], w16["v"][:, cb, :],
                        start=(cb == 0), stop=(cb == CB - 1),
                    )
                nc.vector.tensor_copy(
                    v16[:, c * 4 + tb, 0:H], pja[:, i * 256 : i * 256 + 128]
                )

        p16s = {}

        def emit_pair(c, p):
            t0 = c * 512
            last = p == 2 * c + 1
            sp = pps.tile([128, 1024], F32, tag="sp", name=f"sp_{c}_{p}")
            for ji, off in ((2 * p, 0), (2 * p + 1, 512)):
                d = ji - 4 * c
                # cols left of the diagonal are skipped only on the last
                # (d=2,3) pair, whose exp is range-restricted to match; the
                # d=0,1 pair computes full width so its full-tile exp never
                # reads bytes of the previous PSUM-ring instance.
                q_lo = d * 128 if (last and d > 0) else 0
                nc.tensor.matmul(
                    sp[:, off + q_lo : off + 512],
                    kT16[:, ts(ji, 128)],
                    qT16[:, t0 + q_lo : t0 + 512],
                    start=True, stop=True,
                )
            p16 = pP.tile([128, 1024], dtype, tag="p", name=f"p16_{c}_{p}")
            p16s[(c, p)] = p16
            if last:
                nc.scalar.activation(
                    p16[:, 256:512], sp[:, 256:512],
                    mybir.ActivationFunctionType.Exp, scale=scale,
                )
                nc.scalar.activation(
                    p16[:, 896:1024], sp[:, 896:1024],
                    mybir.ActivationFunctionType.Exp, scale=scale,
                )
            else:
                nc.scalar.activation(
                    p16[:], sp[:],
                    mybir.ActivationFunctionType.Exp, scale=scale,
                )
            for ji, off in ((2 * p, 0), (2 * p + 1, 512)):
                d = ji - 4 * c
                if d >= 0:
                    nc.gpsimd.tensor_mul(
                        p16[:, off + d * 128 : off + (d + 1) * 128],
                        p16[:, off + d * 128 : off + (d + 1) * 128],
                        mask16[:],
                    )

        def scores_chunk(c, hooks):
            for p in range(2 * c + 2):
                emit_pair(c, p)
                for fn in hooks.get(p, ()):
                    fn()

        def av_block(c, qbs):
            # AV accumulation for one or two query blocks; each group
            # completes (and normalizes + stores) within this block, so the
            # shared-pool tile is free for the next block immediately.
            opsT = shr.tile([128, 512], F32, tag="shr", name=f"av_{c}_{qbs[0]}")
            for i, qb in enumerate(qbs):
                i_q = 4 * c + qb
                for ji in range(i_q + 1):
                    off = (ji % 2) * 512
                    nc.tensor.matmul(
                        opsT[:, i * 256 : i * 256 + H + 1],
                        p16s[(c, ji // 2)][:, off + qb * 128 : off + (qb + 1) * 128],
                        v16[:, ji, :],
                        start=(ji == 0), stop=(ji == i_q),
                    )
                sl = opsT[:, i * 256 : i * 256 + H + 1]
                rec = osb.tile([128, 1], F32, tag="rec")
                nc.vector.reciprocal(rec[:], sl[:, H : H + 1])
                o32 = osb.tile([128, H], F32, tag="o32")
                nc.vector.tensor_scalar_mul(o32[:], sl[:, 0:H], rec[:])
                nc.sync.dma_start(
                    out[c * 512 + qb * 128 : c * 512 + (qb + 1) * 128, :],
                    o32[:],
                )

        # ------------------------------------------------------------------
        # Main schedule.  Chunk 0 is special-cased for startup latency; the
        # steady-state loop emits chunk c's pairs with <=2us PE blocks
        # (deferred AV groups, projections for later chunks) hung on fixed
        # hook positions between pairs, then av(c,0) [and av(c,1) for early
        # chunks].  Late chunks defer more AV into the next chunk so no
        # block outruns the exp spine's two banked pairs.
        # ------------------------------------------------------------------
        load_chunk(0)
        load_w("v", 0, CB)
        load_chunk(1)
        load_chunk(2)

        def add_hook(hooks, p, c, fn):
            p = min(p, 2 * c + 1)
            hooks.setdefault(p, []).append(fn)

        qk_proj_interleaved(0)
        scores_chunk(0, {0: [lambda: v_proj(0, (0, 1)),
                             lambda: v_proj(0, (2, 3))]})
        av_block(0, (0, 1))
        proj1(1, "q")
        proj1(1, "k")

        for c in range(1, NCH):
            if c + 2 < NCH:
                load_chunk(c + 2)
            hooks = {}
            pc = c - 1
            # this chunk's own v-projection (needed only by its AV)
            add_hook(hooks, 2, c, lambda cc=c: v_proj(cc, (0, 1)))
            add_hook(hooks, 4, c, lambda cc=c: v_proj(cc, (2, 3)))
            if pc < 2:
                # previous chunk emitted its own av(0,1); its av(2,3) is
                # small enough for one block
                add_hook(hooks, 1, c, lambda pc=pc: av_block(pc, (2, 3)))
            else:
                # previous chunk deferred av(1) too; one block per qb
                add_hook(hooks, 1, c, lambda pc=pc: av_block(pc, (1,)))
                add_hook(hooks, 3, c, lambda pc=pc: av_block(pc, (2,)))
                add_hook(hooks, 5, c, lambda pc=pc: av_block(pc, (3,)))
            if c + 1 < NCH:
                add_hook(hooks, 7, c, lambda cc=c: proj1(cc + 1, "q"))
                add_hook(hooks, 9, c, lambda cc=c: proj1(cc + 1, "k"))
            scores_chunk(c, hooks)
            if c < 2:
                av_block(c, (0, 1))
            else:
                av_block(c, (0,))
        av_block(NCH - 1, (1,))
        av_block(NCH - 1, (2,))
        av_block(NCH - 1, (3,))

    return nc


_NC_CACHE = None


def _get_nc():
    global _NC_CACHE
    if _NC_CACHE is None:
        _apply_tile_patch()
        _NC_CACHE = build_attention()
    return _NC_CACHE


def kernel(x, Wk, Wq, Wv, trace=False):
    """Full inputs in, full output out. Shards batch across the 8 cores."""
    from concourse.bass_utils import run_bass_kernel_spmd

    x = np.asarray(x, dtype=np.float32)
    assert x.shape == (B, T, C), x.shape

    def _warr(w):
        # [C, H] f32 -> [ci, cb*H] fp16 so the on-chip tile loads contiguously
        w16 = np.asarray(w, dtype=np.float32).astype(np.float16)
        return np.ascontiguousarray(
            w16.reshape(C // 128, 128, H).transpose(1, 0, 2).reshape(128, -1)
        )

    Wk16, Wq16, Wv16 = _warr(Wk), _warr(Wq), _warr(Wv)
    # [B,T,C] -> xt[b, chunk, ci, cb*512+t] = x[b, chunk*512+t, cb*128+ci]
    xT16 = np.ascontiguousarray(
        x.transpose(0, 2, 1)
        .astype(np.float16)
        .reshape(B, C // 128, 128, T // 512, 512)
        .transpose(0, 3, 2, 1, 4)
        .reshape(B, T // 512, 128, -1)
    )

    nc = _get_nc()
    in_maps = [
        {"xt": xT16[b], "wq": Wq16, "wk": Wk16, "wv": Wv16} for b in range(B)
    ]
    res = run_bass_kernel_spmd(nc, in_maps, core_ids=list(range(B)), trace=trace)
    outp = np.stack([res.results[b]["out"] for b in range(B)], axis=0)
    if trace:
        global _LAST_RES
        _LAST_RES = res
        return outp, res.exec_time_ns
    return outp
